# revision 24
# baseline (speedup 1.0000x reference)
"""Trainium2 Bass kernel for a single llama-style transformer layer + output head.

Model (per reference):
    h  = rms_norm(x, ln1); q,k,v = proj(h); rope(q, k)
    attn (full, non-causal) per head; x += Wo @ ctx
    h2 = rms_norm(x, ln2); x += Wdown @ (silu(Wgate h2) * (Wup h2))
    logits = x @ W_out.T + b_out            -> reshape(-1, 8, 1024)

Sharding: 8 cores, data-parallel over (batch, seq-half): core c owns batch c//2,
sequence half c%2 (1024 query tokens). Each core computes K/V for its batch's
full 2048-token sequence (small duplicate work) so no collectives are needed.

On-chip convention: activations are FEATURE-MAJOR [d, t] so the contraction
dim of every matmul is the partition dim. Weights are passed pre-transposed
(and pre-tiled where needed) from the host, with the rms-norm gains folded in.
PSUM accumulates in fp32; the residual stream stays fp32 in SBUF.

Perf structure:
  - x loaded once (xs tiles); rms1 stats computed from it; K projection runs
    on RAW x with rstd folded into the rope tables; V/Q run on h = x*rstd.
  - rope shift matmuls are software-pipelined one group behind the
    projection matmuls so the PE never head-of-line blocks on the ACT copy.
  - attention: scores interleave the two heads of a chunk (row-tile halves of
    the PE), exp is split between ACT (true exp) and DVE (fast exp2 via fp8
    bit arithmetic), and AV uses fp8 DoubleRow over key-tile pairs.
  - MLP optionally runs entirely in fp8 with DoubleRow (mlp_fp8 flag).
"""

import dataclasses
import math

import numpy as np
import ml_dtypes

import concourse.bass as bass
import concourse.bacc as bacc
import concourse.tile as tile
import concourse.mybir as mybir
from concourse import bass_utils
from concourse.alu_op_type import AluOpType

BF16 = mybir.dt.bfloat16
F32 = mybir.dt.float32
FP8 = mybir.dt.float8e4
U8 = mybir.dt.uint8
AF = mybir.ActivationFunctionType
DR = mybir.MatmulPerfMode.DoubleRow
NPBF = ml_dtypes.bfloat16
NPF8 = ml_dtypes.float8_e4m3

N_CORES = 8

# fast exp2-on-bits constants: byte(fp8e4m3(exp(s))) ~= s*8/ln2 + 56
EXPA = 8.0 / math.log(2.0)
EXPB = 56.0


@dataclasses.dataclass(frozen=True)
class Cfg:
    D: int = 1024      # model dim
    S: int = 2048      # full seq (per batch)
    TQ: int = 1024     # query tokens per core
    H: int = 16        # heads
    HD: int = 64       # head dim
    FF: int = 4096     # mlp intermediate
    V: int = 1024      # output head size
    NT: int = 512      # matmul moving-dim tile
    EPS: float = 1e-6
    THETA: float = 10000.0
    av_dr: bool = True     # AV matmul via fp8 DoubleRow (kt pairs)
    mlp_fp8: int = 1       # 0=bf16 MLP, 1=down-proj fp8 DR, 2=full fp8 DR
    exp_dve: bool = True   # offload half the exps to DVE (fast exp trick)

    @property
    def CD(self):
        return self.D // 128

    @property
    def CF(self):
        return self.FF // 128

    @property
    def KT(self):
        return self.S // 128

    @property
    def TT(self):
        return self.TQ // 128

    @property
    def HPC(self):
        return 128 // self.HD  # heads per 128-partition chunk (2)


FULL = Cfg()


def _nt_slices(total, nt):
    return [(i * nt, nt) for i in range(total // nt)]


def build_bass(cfg: Cfg):
    """Build the SPMD Bass program. Returns nc."""
    c = cfg
    nc = bacc.Bacc("TRN2", target_bir_lowering=False, debug=False,
                   num_devices=N_CORES)

    # register an eps const AP (activation() converts float biases to APs)
    _eps_t = nc.alloc_sbuf_tensor("const-eps", [128, 1], F32)
    nc.gpsimd.memset(_eps_t.ap(), c.EPS)
    nc.const_aps.aps[(F32, c.EPS)] = _eps_t.ap()

    dt = nc.dram_tensor
    x_fm = dt("x_fm", [c.D, c.S], BF16, kind="ExternalInput").ap()
    x_own = dt("x_own", [c.D, c.TQ], F32, kind="ExternalInput").ap()
    wqT = dt("wqT", [c.CD, 128, c.CD * 128], BF16, kind="ExternalInput").ap()
    wkT = dt("wkT", [c.CD, 128, c.CD * 128], BF16, kind="ExternalInput").ap()
    woT = dt("woT", [c.CD, 128, c.CD * 128], BF16, kind="ExternalInput").ap()
    wvT = dt("wvT", [c.D, c.D], BF16, kind="ExternalInput").ap()
    _W = min(512, c.FF)
    _n_fog = max(1, c.FF // 512)
    _KQ = c.CD // 2  # contraction pair count for D-dim (DoubleRow)
    _FQ = c.CF // 2  # contraction pair count for FF-dim
    if c.mlp_fp8 >= 2:
        wgT = dt("wgT", [_n_fog, 128, _KQ * 2 * _W], FP8, kind="ExternalInput").ap()
        wuT = dt("wuT", [_n_fog, 128, _KQ * 2 * _W], FP8, kind="ExternalInput").ap()
    else:
        wgT = dt("wgT", [_n_fog, 128, c.CD * _W], BF16, kind="ExternalInput").ap()
        wuT = dt("wuT", [_n_fog, 128, c.CD * _W], BF16, kind="ExternalInput").ap()
    if c.mlp_fp8 >= 1:
        wdT = dt("wdT", [c.CD, 128, _FQ * 2 * 128], FP8, kind="ExternalInput").ap()
    else:
        wdT = dt("wdT", [c.CD, 128, c.CF * 128], BF16, kind="ExternalInput").ap()
    woutT = dt("woutT", [c.D, c.V], BF16, kind="ExternalInput").ap()
    bias_row = dt("bias_row", [1, c.V], BF16, kind="ExternalInput").ap()
    cos_s = dt("cos_s", [128, c.S], BF16, kind="ExternalInput").ap()
    sin_s = dt("sin_s", [128, c.S], BF16, kind="ExternalInput").ap()
    shiftT = dt("shiftT", [128, 128], BF16, kind="ExternalInput").ap()
    sel = dt("sel", [c.H, c.D], BF16, kind="ExternalInput").ap()
    onesb_d = dt("onesb", [128, 128], BF16, kind="ExternalInput").ap()

    logits = dt("logits", [c.TQ, c.V], F32, kind="ExternalOutput").ap()

    halves = c.S // c.TQ
    _pid = nc.partition_id()
    qoff = (_pid % halves) * c.TQ

    with tile.TileContext(nc) as tc:
        # ---------- small whole-kernel constants ----------
        const = tc.alloc_tile_pool(name="const", bufs=1)
        ones_b = const.tile([128, 128], BF16)
        nc.sync.dma_start(ones_b[:], onesb_d[:])
        shift_sb = const.tile([128, 128], BF16)
        nc.sync.dma_start(shift_sb[:], shiftT[:])

        # ---------- right-side stack: long-lived cross-phase tensors ----------
        p_ctxn = tc.alloc_tile_pool(name="ctxn", bufs=1, side="right")
        ctxn = [p_ctxn.tile([128, c.TQ], BF16, name=f"ctxn{i}") for i in range(c.CD)]
        p_den = tc.alloc_tile_pool(name="den", bufs=1, side="right")
        den_sb = p_den.tile([c.H, c.TQ], F32)
        sel_sb = p_den.tile([c.H, c.D], BF16)
        nc.sync.dma_start(sel_sb[:], sel[:])
        bias_sb = p_den.tile([1, c.V], BF16)
        nc.sync.dma_start(bias_sb[:], bias_row[:])

        # ---------- left: K/V (+ Q later) outputs, span B -> C ----------
        p_kv = tc.alloc_tile_pool(name="kv", bufs=1)
        kr = [p_kv.tile([128, c.S], BF16, name=f"kr{i}") for i in range(c.CD)]
        # V token-major in fp8, kt-PAIRED for DoubleRow: vt2[j][:, par, :] is
        # the 128-token block to = 2j+par; within a block: one 128-wide group
        # per head, cols [0:HD)=V, col HD=ones (denominator trick), rest 0.
        vt2 = [p_kv.tile([128, 2, c.H * 128], FP8, name=f"vt2_{j}")
               for j in range(c.KT // 2)]
        vtf = [t.rearrange("p a b -> p (a b)") for t in vt2]
        p_qr = tc.alloc_tile_pool(name="qr", bufs=1)
        qr = [p_qr.tile([128, c.TQ], BF16, name=f"qr{i}") for i in range(c.CD)]

        # =======================================================
        # PHASE A: load x once; rms1 stats; rstd folded into tables
        # =======================================================
        pA = tc.alloc_tile_pool(name="phA", bufs=1)
        xs = [pA.tile([128, c.S], BF16, name=f"xs{i}") for i in range(c.CD)]
        for cd in range(c.CD):
            nc.sync.dma_start(xs[cd][:], x_fm[cd * 128:(cd + 1) * 128, :])

        pB_w = tc.alloc_tile_pool(name="phB_w", bufs=2)
        cos_s_sb = pB_w.tile([128, c.S], BF16, name="cos_s_sb", bufs=1)
        nc.sync.dma_start(cos_s_sb[:], cos_s[:])
        sin_s_sb = pB_w.tile([128, c.S], BF16, name="sin_s_sb", bufs=1)
        nc.sync.dma_start(sin_s_sb[:], sin_s[:])
        # full V weight resident (16KB/part), prefetched during stats
        wv_full = pB_w.tile([128, c.CD * c.D], BF16, name="wv_full", bufs=1)
        for kc in range(c.CD):
            nc.sync.dma_start(wv_full[:, kc * c.D:(kc + 1) * c.D],
                              wvT[kc * 128:(kc + 1) * 128, :])
        pA_t = tc.alloc_tile_pool(name="phA_t", bufs=1)
        rstd = pA_t.tile([1, c.S], BF16, name="rstd")
        rstd_col = pA_t.tile([128, c.KT], F32, name="rstd_col")
        pA_s = tc.alloc_tile_pool(name="phA_s", bufs=3)

        pA_ss = tc.alloc_tile_pool(name="phA_ss", bufs=1, space="PSUM")
        ss = {o: pA_ss.tile([1, c.NT], F32, name=f"ss{o}")
              for (o, n) in _nt_slices(c.S, c.NT)}
        for cd in range(c.CD):
            for (o, n) in _nt_slices(c.S, c.NT):
                sq = pA_s.tile([128, c.NT], BF16, tag="sq", bufs=3)
                nc.vector.tensor_tensor(sq[:], xs[cd][:, o:o + n],
                                        xs[cd][:, o:o + n], op=AluOpType.mult)
                nc.tensor.matmul(ss[o][:], ones_b[:, 0:1], sq[:],
                                 start=(cd == 0), stop=(cd == c.CD - 1))
        # rsqrt(m) = exp(-0.5 * ln(m))
        for (o, n) in _nt_slices(c.S, c.NT):
            nc.scalar.activation(rstd[:, o:o + n], ss[o][:], AF.Ln,
                                 bias=c.EPS, scale=1.0 / c.D)
        nc.scalar.activation(rstd[:], rstd[:], AF.Exp, scale=-0.5)
        pA_ss.release()

        # broadcast rstd over partitions (PE outer product); fold it into the
        # rope tables IN PLACE (K and Q both project raw x), and transpose it
        # into per-token-block columns for the V scaling.
        p_rb = tc.alloc_tile_pool(name="p_rb", bufs=1)
        rb_sb = p_rb.tile([128, c.S], BF16, name="rb_sb")
        pA_rb = tc.alloc_tile_pool(name="phA_rb", bufs=2, space="PSUM")
        for (o, n) in _nt_slices(c.S, c.NT):
            rbt = pA_rb.tile([128, c.NT], F32, tag="rb")
            nc.tensor.matmul(rbt[:], ones_b[0:1, :], rstd[:, o:o + n],
                             start=True, stop=True)
            nc.vector.tensor_copy(rb_sb[:, o:o + n], rbt[:])
        for tt in range(c.KT):
            tp = pA_rb.tile([128, 1], BF16, tag="tp")
            nc.tensor.transpose(tp[:], rstd[0:1, tt * 128:(tt + 1) * 128],
                                ones_b[0:1, 0:1])
            nc.vector.tensor_copy(rstd_col[:, tt:tt + 1], tp[:])
        pA_rb.release()
        for (o, n) in _nt_slices(c.S, c.NT):
            nc.vector.tensor_tensor(cos_s_sb[:, o:o + n], rb_sb[:, o:o + n],
                                    cos_s_sb[:, o:o + n], op=AluOpType.mult)
            nc.vector.tensor_tensor(sin_s_sb[:, o:o + n], rb_sb[:, o:o + n],
                                    sin_s_sb[:, o:o + n], op=AluOpType.mult)
        p_rb.release()

        # =======================================================
        # PHASE B: K proj (from raw x) + rope, V proj (from h,
        # token-major kt-paired), then Q (from h) + rope
        # =======================================================
        pB_t = tc.alloc_tile_pool(name="phB_t", bufs=2)
        pB_ps = tc.alloc_tile_pool(name="phB_ps", bufs=4, space="PSUM")
        pB_ps2 = tc.alloc_tile_pool(name="phB_ps2", bufs=4, space="PSUM")

        def rope_combine(pool, raw, psk, cos_ap, sin_ap, n, dst):
            """dst = raw*cos + (S@raw)*sin, all [128, n]."""
            t1 = pool.tile([128, c.NT], BF16, tag="ropet1", bufs=4)
            nc.vector.tensor_tensor(t1[:, 0:n], raw[:], cos_ap,
                                    op=AluOpType.mult)
            t2 = pool.tile([128, c.NT], BF16, tag="ropet2", bufs=4)
            nc.vector.tensor_tensor(t2[:, 0:n], psk[:], sin_ap,
                                    op=AluOpType.mult)
            nc.vector.tensor_tensor(dst[:], t1[:, 0:n], t2[:, 0:n],
                                    op=AluOpType.add)

        def emit_shift_rope(raw, o, n, dst_t, cos_t, sin_t, dyn_off):
            psk = pB_ps2.tile([128, c.NT], F32, tag="pshift")
            nc.tensor.matmul(psk[:, 0:n], shift_sb[:], raw[:, 0:n],
                             start=True, stop=True)
            if dyn_off is None:
                cos_ap = cos_t[:, o:o + n]
                sin_ap = sin_t[:, o:o + n]
            else:
                cos_ap = cos_t[:, bass.ds(dyn_off + o, n)]
                sin_ap = sin_t[:, bass.ds(dyn_off + o, n)]
            rope_combine(pB_t, raw[:, 0:n], psk[:, 0:n], cos_ap, sin_ap, n,
                         dst_t[:, o:o + n])

        # --- K projection + rope (rstd folded into the tables) ---
        pend = None
        for mo in range(c.CD):
            wk_t = pB_w.tile([128, c.CD * 128], BF16, tag="wk")
            nc.sync.dma_start(
                wk_t[:], wkT[mo:mo + 1].rearrange("o p f -> (o p) f"))
            for (o, n) in _nt_slices(c.S, c.NT):
                pk = pB_ps.tile([128, c.NT], F32, tag="pproj")
                for kc in range(c.CD):
                    nc.tensor.matmul(pk[:], wk_t[:, kc * 128:(kc + 1) * 128],
                                     xs[kc][:, o:o + n],
                                     start=(kc == 0), stop=(kc == c.CD - 1))
                raw = pB_t.tile([128, c.NT], BF16, tag="kraw", bufs=8)
                nc.scalar.copy(raw[:, 0:n], pk[:])
                if pend is not None:
                    emit_shift_rope(*pend)
                pend = (raw, o, n, kr[mo], cos_s_sb, sin_s_sb, None)
        emit_shift_rope(*pend)
        pend = None

        # --- V projection (raw x, token-major kt-paired, rstd at the copy,
        # ones column for the denominator) ---
        for to in range(c.KT):
            j, par = to // 2, to % 2
            for (o, n) in _nt_slices(c.D, c.NT):
                pv = pB_ps.tile([128, c.NT], F32, tag="pproj")
                for kc in range(c.CD):
                    nc.tensor.matmul(pv[:], xs[kc][:, to * 128:(to + 1) * 128],
                                     wv_full[:, kc * c.D + o: kc * c.D + o + n],
                                     start=(kc == 0), stop=(kc == c.CD - 1))
                nh = n // c.HD
                h0 = o // c.HD
                base = par * (c.H * 128) + h0 * 128
                dstv = vtf[j][:, base: base + nh * 128].rearrange(
                    "p (h e) -> p h e", e=128)
                nc.vector.tensor_scalar(
                    dstv[:, :, 0:c.HD],
                    pv.rearrange("p (h e) -> p h e", e=c.HD),
                    rstd_col[:, to:to + 1], None, op0=AluOpType.mult)
        for j in range(c.KT // 2):
            for par in range(2):
                dstv = vtf[j][:, par * c.H * 128:(par + 1) * c.H * 128].rearrange(
                    "p (h e) -> p h e", e=128)
                nc.gpsimd.memset(dstv[:, :, c.HD:c.HD + 1], 1.0)
                nc.gpsimd.memset(dstv[:, :, c.HD + 1:], 0.0)

        # --- Q: projection from raw x own-half + rope (folded tables) ---
        for mo in range(c.CD):
            wq_t = pB_w.tile([128, c.CD * 128], BF16, tag="wk")
            nc.sync.dma_start(
                wq_t[:], wqT[mo:mo + 1].rearrange("o p f -> (o p) f"))
            for (o, n) in _nt_slices(c.TQ, c.NT):
                pq = pB_ps.tile([128, c.NT], F32, tag="pproj")
                for kc in range(c.CD):
                    nc.tensor.matmul(pq[:], wq_t[:, kc * 128:(kc + 1) * 128],
                                     xs[kc][:, bass.ds(qoff + o, n)],
                                     start=(kc == 0), stop=(kc == c.CD - 1))
                raw = pB_t.tile([128, c.NT], BF16, tag="kraw", bufs=8)
                nc.scalar.copy(raw[:, 0:n], pq[:])
                if pend is not None:
                    emit_shift_rope(*pend)
                pend = (raw, o, n, qr[mo], cos_s_sb, sin_s_sb, qoff)
        emit_shift_rope(*pend)
        pend = None

        pB_ps2.release()
        pB_ps.release()
        pB_t.release()
        pA_s.release()
        pA_t.release()
        pB_w.release()
        pA.release()       # raw x freed

        # =======================================================
        # PHASE C: attention (ctxn holds unnormalized ctx, then
        # normalized in place)
        # =======================================================
        pC_exp = tc.alloc_tile_pool(name="phC_exp", bufs=1)
        pC_sc = tc.alloc_tile_pool(name="phC_sc", bufs=1, space="PSUM")
        pC_av = tc.alloc_tile_pool(name="phC_av", bufs=2, space="PSUM")
        expctr = [0]

        def emit_scores_pair(hp):
            # scores for the two heads of chunk hp; one [128, TQ] psum per
            # (kt, sl) so the stationary loads once per 2 matmuls; exp split
            # ACT/DVE, one activation per psum tile
            ch = hp
            tiles = {}
            for kt in range(c.KT):
                j, par = kt // 2, kt % 2
                for (o, n) in _nt_slices(c.TQ, c.NT):
                    for sl in range(c.HPC):
                        poff = sl * c.HD
                        sc = pC_sc.tile([128, c.NT], F32, tag=f"sc{sl}", bufs=3)
                        nc.tensor.matmul(
                            sc[:, 0:n],
                            kr[ch][poff:poff + c.HD, kt * 128:(kt + 1) * 128],
                            qr[ch][poff:poff + c.HD, o:o + n],
                            start=True, stop=True)
                        if (sl, j) not in tiles:
                            tiles[(sl, j)] = pC_exp.tile(
                                [128, 2, c.TQ], FP8, tag=f"e{sl}_{j}", bufs=2,
                                name=f"e{sl}_{j}")
                        ef = tiles[(sl, j)].rearrange("p a b -> p (a b)")
                        dst = ef[:, par * c.TQ + o: par * c.TQ + o + n]
                        expctr[0] += 1
                        if c.exp_dve and (expctr[0] % 2 == 0):
                            nc.vector.tensor_scalar(
                                dst.bitcast(U8), sc[:, 0:n], EXPA, EXPB,
                                op0=AluOpType.mult, op1=AluOpType.add)
                        else:
                            nc.scalar.activation(dst, sc[:, 0:n], AF.Exp)
            return tiles

        def emit_av_pair(hp, tiles):
            ch = hp
            for sl in range(c.HPC):
                hh = hp * c.HPC + sl
                poff = sl * c.HD
                ost = _nt_slices(c.TQ, c.NT)
                avs = [pC_av.tile([128, c.NT], F32, tag="av", name=f"av{oi}")
                       for oi in range(len(ost))]
                # o-inner so each vt2 stationary loads once per len(ost) MMs
                if c.av_dr:
                    for j in range(c.KT // 2):
                        for oi, (o, n) in enumerate(ost):
                            nc.tensor.matmul(
                                avs[oi][:, 0:n],
                                vt2[j][:, :, hh * 128:(hh + 1) * 128],
                                tiles[(sl, j)][:, :, o:o + n],
                                start=(j == 0), stop=(j == c.KT // 2 - 1),
                                perf_mode=DR)
                else:
                    for kt in range(c.KT):
                        j, par = kt // 2, kt % 2
                        vsl = vtf[j][:, par * c.H * 128 + hh * 128:
                                     par * c.H * 128 + (hh + 1) * 128]
                        for oi, (o, n) in enumerate(ost):
                            esl = tiles[(sl, j)].rearrange("p a b -> p (a b)")[
                                :, par * c.TQ + o: par * c.TQ + o + n]
                            nc.tensor.matmul(avs[oi][:, 0:n], vsl, esl,
                                             start=(kt == 0),
                                             stop=(kt == c.KT - 1))
                for oi, (o, n) in enumerate(ost):
                    av = avs[oi]
                    nc.vector.tensor_copy(ctxn[ch][poff:poff + c.HD, o:o + n],
                                          av[0:c.HD, 0:n])
                    # den row sits at psum partition HD(=64); engines cannot
                    # move it to partition hh: stage in SBUF, DMA-gather.
                    dstage = pC_exp.tile([128, c.NT], F32, tag="dstage", bufs=3)
                    nc.vector.tensor_copy(dstage[c.HD:c.HD + 1, :],
                                          av[c.HD:c.HD + 1, :])
                    nc.sync.dma_start(den_sb[hh:hh + 1, o:o + n],
                                      dstage[c.HD:c.HD + 1, :])

        # software-pipeline head pairs: scores(p+1) emitted before AV(p) so
        # the PE always has exp-independent matmul work while ACT/DVE run exp
        prev = None
        for hp in range(c.H // c.HPC):
            et = emit_scores_pair(hp)
            if prev is not None:
                emit_av_pair(*prev)
            prev = (hp, et)
        emit_av_pair(*prev)

        # 1/x = exp(-ln(x));  recip lands in bf16 for the sel matmul
        nc.scalar.activation(den_sb[:], den_sb[:], AF.Ln)
        recip_bf = p_den.tile([c.H, c.TQ], BF16, name="recip_bf")
        nc.scalar.activation(recip_bf[:], den_sb[:], AF.Exp, scale=-1.0)
        pC_av.release()
        pC_sc.release()
        pC_rb = tc.alloc_tile_pool(name="phC_rb", bufs=2, space="PSUM")
        for ch in range(c.CD):
            for (o, n) in _nt_slices(c.TQ, c.NT):
                prb = pC_rb.tile([128, c.NT], F32, tag="prb")
                nc.tensor.matmul(prb[:], sel_sb[:, ch * 128:(ch + 1) * 128],
                                 recip_bf[:, o:o + n], start=True, stop=True)
                nc.vector.tensor_tensor(ctxn[ch][:, o:o + n], ctxn[ch][:, o:o + n],
                                        prb[:], op=AluOpType.mult)

        pC_rb.release()
        pC_exp.release()
        p_qr.release()
        p_kv.release()

        # =======================================================
        # PHASE D: Wo proj + residual, rms2, h2   (right-side pool)
        # =======================================================
        pD = tc.alloc_tile_pool(name="phD", bufs=1, side="right")
        xo2 = [pD.tile([128, c.TQ], F32, name=f"xo2_{i}") for i in range(c.CD)]
        if c.mlp_fp8 >= 2:
            h2f = [pD.tile([128, 2, c.TQ], FP8, name=f"h2f_{i}")
                   for i in range(_KQ)]
        else:
            h2 = [pD.tile([128, c.TQ], BF16, name=f"h2_{i}") for i in range(c.CD)]
        h3 = [pD.tile([128, c.TQ], BF16, name=f"h3_{i}") for i in range(c.CD)]
        pD_w = tc.alloc_tile_pool(name="phD_w", bufs=3)
        pD_t = tc.alloc_tile_pool(name="phD_t", bufs=3)
        pD_ps = tc.alloc_tile_pool(name="phD_ps", bufs=3, space="PSUM")
        pD_ps1 = tc.alloc_tile_pool(name="phD_ps1", bufs=1, space="PSUM")

        ss2 = {o: pD_ps1.tile([1, c.NT], F32, name=f"ss2_{o}")
               for (o, n) in _nt_slices(c.TQ, c.NT)}
        for mo in range(c.CD):
            wo_t = pD_w.tile([128, c.CD * 128], BF16, tag="wo")
            nc.sync.dma_start(
                wo_t[:], woT[mo:mo + 1].rearrange("o p f -> (o p) f"))
            xot = pD_t.tile([128, c.TQ], F32, tag="xot")
            nc.sync.dma_start(xot[:], x_own[mo * 128:(mo + 1) * 128, :])
            for (o, n) in _nt_slices(c.TQ, c.NT):
                po = pD_ps.tile([128, c.NT], F32, tag="po")
                for kc in range(c.CD):
                    nc.tensor.matmul(po[:], wo_t[:, kc * 128:(kc + 1) * 128],
                                     ctxn[kc][:, o:o + n],
                                     start=(kc == 0), stop=(kc == c.CD - 1))
                nc.vector.tensor_tensor(xo2[mo][:, o:o + n], xot[:, o:o + n],
                                        po[:], op=AluOpType.add)
                sq = pD_t.tile([128, c.NT], BF16, tag="sq2")
                nc.scalar.activation(sq[:], xo2[mo][:, o:o + n], AF.Square)
                nc.tensor.matmul(ss2[o][:], ones_b[:, 0:1], sq[:],
                                 start=(mo == 0), stop=(mo == c.CD - 1))
        rstd2 = pD_t.tile([1, c.TQ], BF16, tag="rstd2", bufs=1)
        for (o, n) in _nt_slices(c.TQ, c.NT):
            nc.scalar.activation(rstd2[:, o:o + n], ss2[o][:], AF.Ln,
                                 bias=c.EPS, scale=1.0 / c.D)
        nc.scalar.activation(rstd2[:], rstd2[:], AF.Exp, scale=-0.5)
        for (o, n) in _nt_slices(c.TQ, c.NT):
            rbt = pD_ps.tile([128, c.NT], F32, tag="po")
            nc.tensor.matmul(rbt[:], ones_b[0:1, :], rstd2[:, o:o + n],
                             start=True, stop=True)
            for cd in range(c.CD):
                if c.mlp_fp8 >= 2:
                    hf = h2f[cd // 2].rearrange("p a b -> p (a b)")
                    dst = hf[:, (cd % 2) * c.TQ + o: (cd % 2) * c.TQ + o + n]
                else:
                    dst = h2[cd][:, o:o + n]
                nc.vector.tensor_tensor(dst, xo2[cd][:, o:o + n],
                                        rbt[:], op=AluOpType.mult)

        pD_ps1.release()
        pD_ps.release()
        pD_t.release()
        pD_w.release()

        # =======================================================
        # PHASE E: MLP (swiglu), t-tile outer loop
        # =======================================================
        # output-head pools (head tiles are emitted inside the E loop per
        # t-half, as soon as that half's h3 chunks are complete)
        pF = tc.alloc_tile_pool(name="phF", bufs=1)
        wout_t = pF.tile([128, c.CD * c.V], BF16)
        for kc in range(c.CD):
            nc.sync.dma_start(wout_t[:, kc * c.V:(kc + 1) * c.V],
                              woutT[kc * 128:(kc + 1) * 128, :])
        pF_t = tc.alloc_tile_pool(name="phF_t", bufs=3)
        pF_ps = tc.alloc_tile_pool(name="phF_ps", bufs=2, space="PSUM")

        def emit_head(to):
            for (o, n) in _nt_slices(c.V, c.NT):
                ph = pF_ps.tile([128, c.NT], F32, tag="ph")
                for kc in range(c.CD):
                    nc.tensor.matmul(ph[:], h3[kc][:, to * 128:(to + 1) * 128],
                                     wout_t[:, kc * c.V + o: kc * c.V + o + n],
                                     start=(kc == 0), stop=False)
                nc.tensor.matmul(ph[:], ones_b[0:1, :], bias_sb[:, o:o + n],
                                 start=False, stop=True)
                lg = pF_t.tile([128, c.NT], F32, tag="lg")
                nc.vector.tensor_copy(lg[:], ph[:])
                nc.sync.dma_start(logits[to * 128:(to + 1) * 128, o:o + n], lg[:])

        pE = tc.alloc_tile_pool(name="phE", bufs=1)
        if c.mlp_fp8 >= 1:
            gu2 = [pE.tile([128, 2, c.NT], FP8, name=f"gu2_{i}")
                   for i in range(_FQ)]
        else:
            gu = [pE.tile([128, c.NT], BF16, name=f"gu{i}") for i in range(c.CF)]
        pE_w = tc.alloc_tile_pool(name="phE_w", bufs=2)
        pE_t = tc.alloc_tile_pool(name="phE_t", bufs=3)
        pE_ps = tc.alloc_tile_pool(name="phE_ps", bufs=2, space="PSUM")

        W = _W
        n_fog = _n_fog
        fpg = c.CF // n_fog  # fo chunks per group (128-wide ff blocks)
        for (o, n) in _nt_slices(c.TQ, c.NT):
            for fg in range(n_fog):
                if c.mlp_fp8 >= 2:
                    wg_t = pE_w.tile([128, _KQ * 2 * W], FP8, tag="wg")
                    wu_t = pE_w.tile([128, _KQ * 2 * W], FP8, tag="wu")
                else:
                    wg_t = pE_w.tile([128, c.CD * W], BF16, tag="wg")
                    wu_t = pE_w.tile([128, c.CD * W], BF16, tag="wu")
                nc.sync.dma_start(wg_t[:], wgT[fg:fg + 1].rearrange("o p f -> (o p) f"))
                nc.sync.dma_start(wu_t[:], wuT[fg:fg + 1].rearrange("o p f -> (o p) f"))
                for fi in range(fpg):
                    fo = fg * fpg + fi
                    pg = pE_ps.tile([128, c.NT], F32, tag="pg")
                    pu = pE_ps.tile([128, c.NT], F32, tag="pu")
                    if c.mlp_fp8 >= 2:
                        for kq in range(_KQ):
                            lsl = wg_t[:, kq * 2 * W: (kq + 1) * 2 * W].rearrange(
                                "p (i j) -> p i j", i=2)[:, :, fi * 128:(fi + 1) * 128]
                            nc.tensor.matmul(
                                pg[:, 0:n], lsl, h2f[kq][:, :, o:o + n],
                                start=(kq == 0), stop=(kq == _KQ - 1),
                                perf_mode=DR)
                        for kq in range(_KQ):
                            lsl = wu_t[:, kq * 2 * W: (kq + 1) * 2 * W].rearrange(
                                "p (i j) -> p i j", i=2)[:, :, fi * 128:(fi + 1) * 128]
                            nc.tensor.matmul(
                                pu[:, 0:n], lsl, h2f[kq][:, :, o:o + n],
                                start=(kq == 0), stop=(kq == _KQ - 1),
                                perf_mode=DR)
                    else:
                        for kc in range(c.CD):
                            nc.tensor.matmul(
                                pg[:, 0:n],
                                wg_t[:, kc * W + fi * 128: kc * W + (fi + 1) * 128],
                                h2[kc][:, o:o + n],
                                start=(kc == 0), stop=(kc == c.CD - 1))
                        for kc in range(c.CD):
                            nc.tensor.matmul(
                                pu[:, 0:n],
                                wu_t[:, kc * W + fi * 128: kc * W + (fi + 1) * 128],
                                h2[kc][:, o:o + n],
                                start=(kc == 0), stop=(kc == c.CD - 1))
                    if c.mlp_fp8 >= 1:
                        g = pE_t.tile([128, c.NT], BF16, tag="g")
                        nc.scalar.activation(g[:, 0:n], pg[:, 0:n], AF.Silu)
                        guf = gu2[fo // 2].rearrange("p a b -> p (a b)")
                        dst = guf[:, (fo % 2) * c.NT: (fo % 2) * c.NT + n]
                        nc.vector.tensor_tensor(dst, g[:, 0:n], pu[:, 0:n],
                                                op=AluOpType.mult)
                    else:
                        g = pE_t.tile([128, c.NT], BF16, tag="g")
                        nc.scalar.activation(g[:, 0:n], pg[:, 0:n], AF.Silu)
                        nc.vector.tensor_tensor(gu[fo][:, 0:n], g[:, 0:n], pu[:, 0:n],
                                                op=AluOpType.mult)
            # down proj + residual -> h3 (bf16)
            for mo in range(c.CD):
                if c.mlp_fp8 >= 1:
                    wd_t = pE_w.tile([128, _FQ * 2 * 128], FP8, tag="wd")
                else:
                    wd_t = pE_w.tile([128, c.CF * 128], BF16, tag="wd")
                nc.sync.dma_start(
                    wd_t[:], wdT[mo:mo + 1].rearrange("o p f -> (o p) f"))
                pd = pE_ps.tile([128, c.NT], F32, tag="pd")
                if c.mlp_fp8 >= 1:
                    for fq in range(_FQ):
                        lsl = wd_t[:, fq * 256:(fq + 1) * 256].rearrange(
                            "p (i f) -> p i f", i=2)
                        nc.tensor.matmul(pd[:, 0:n], lsl, gu2[fq][:, :, 0:n],
                                         start=(fq == 0), stop=(fq == _FQ - 1),
                                         perf_mode=DR)
                else:
                    for fc in range(c.CF):
                        nc.tensor.matmul(pd[:, 0:n], wd_t[:, fc * 128:(fc + 1) * 128],
                                         gu[fc][:, 0:n],
                                         start=(fc == 0), stop=(fc == c.CF - 1))
                nc.vector.tensor_tensor(h3[mo][:, o:o + n], xo2[mo][:, o:o + n],
                                        pd[:, 0:n], op=AluOpType.add)
            for to in range(o // 128, (o + n) // 128):
                emit_head(to)

        pE_ps.release()
        pE_t.release()
        pE_w.release()
        pE.release()
        pF_ps.release()
        pF_t.release()
        pF.release()
        pD.release()
        p_den.release()
        p_ctxn.release()
        const.release()

    nc.compile()
    return nc


# ===================== host side =====================

def _bf(a):
    return np.ascontiguousarray(np.asarray(a, dtype=np.float32)).astype(NPBF)


def _f8(a):
    return np.ascontiguousarray(np.asarray(a, dtype=np.float32)).astype(NPF8)


def make_tables(c: Cfg):
    pos = np.arange(c.S, dtype=np.float32)
    inv = 1.0 / (c.THETA ** (np.arange(0, c.HD, 2, dtype=np.float32) / c.HD))
    ang = pos[:, None] * inv[None, :]                      # [S, HD/2]
    cos = np.concatenate([np.cos(ang), np.cos(ang)], -1).T  # [HD, S]
    sin = np.concatenate([np.sin(ang), np.sin(ang)], -1).T
    sign = np.where(np.arange(c.HD) < c.HD // 2, -1.0, 1.0)[:, None].astype(np.float32)
    cos_t = _bf(np.tile(cos, (c.HPC, 1)))                  # [128, S]
    sin_t = _bf(np.tile(sin * sign, (c.HPC, 1)))

    shiftT = np.zeros((128, 128), dtype=np.float32)
    for m in range(128):
        src = m + 32 if (m % c.HD) < c.HD // 2 else m - 32
        shiftT[src, m] = 1.0
    sel = np.zeros((c.H, c.D), dtype=np.float32)
    for ch in range(c.CD):
        for m in range(128):
            sel[ch * c.HPC + m // c.HD, ch * 128 + m] = 1.0
    return cos_t, sin_t, _bf(shiftT), _bf(sel)


def tile_lhsT(wT):
    """[K, M] -> [M/128 (mo), 128 (p), K (kc*128+f)] packed lhsT rows.

    out[mo, p, kc*128+f] = wT[kc*128+p, mo*128+f] so one contiguous DMA
    yields the SBUF tile whose [:, kc*128:(kc+1)*128] slice is the
    [K=128, M=128] stationary block for contraction chunk kc.
    """
    K, M = wT.shape
    t = wT.reshape(K // 128, 128, M // 128, 128)       # [kc, p, mo, f]
    return np.ascontiguousarray(t.transpose(2, 1, 0, 3).reshape(M // 128, 128, K))


def tile_lhsT_dr8(wT):
    """[K, M] -> [M/128, 128, (K/256)*2*128] fp8 DoubleRow-paired lhsT.

    out[mo, p, kq*256 + i*128 + f] = wT[kq*256 + i*128 + p, mo*128 + f]
    """
    K, M = wT.shape
    t = np.asarray(wT, dtype=np.float32).reshape(K // 256, 2, 128, M // 128, 128)
    return _f8(t.transpose(3, 2, 0, 1, 4).reshape(M // 128, 128, K))


def tile_fog(wT, W):
    """[D, FF] -> [FF/W (fg), 128 (p), (D/128)*W] packed gate/up slices."""
    D, FF = wT.shape
    t = wT.reshape(D // 128, 128, FF // W, W)          # [kc, p, fg, j]
    return np.ascontiguousarray(
        t.transpose(2, 1, 0, 3).reshape(FF // W, 128, D // 128 * W))


def tile_fog_dr8(wT, W):
    """[D, FF] -> [FF/W, 128, (D/256)*2*W] fp8 DoubleRow-paired gate/up.

    out[fg, p, kq*2W + i*W + j] = wT[kq*256 + i*128 + p, fg*W + j]
    """
    D, FF = wT.shape
    t = np.asarray(wT, dtype=np.float32).reshape(D // 256, 2, 128, FF // W, W)
    return _f8(t.transpose(3, 2, 0, 1, 4).reshape(FF // W, 128, (D // 256) * 2 * W))


def prep_in_maps(c: Cfg, inputs: dict, n_cores: int = N_CORES):
    x = np.asarray(inputs["chunk_hidden_states"], dtype=np.float32)  # [B,S,D]
    ln1 = np.asarray(inputs["ln1_w"], dtype=np.float32)
    ln2 = np.asarray(inputs["ln2_w"], dtype=np.float32)
    wq = np.asarray(inputs["Wq"], dtype=np.float32)
    wk = np.asarray(inputs["Wk"], dtype=np.float32)
    wv = np.asarray(inputs["Wv"], dtype=np.float32)
    wo = np.asarray(inputs["Wo"], dtype=np.float32)
    wg = np.asarray(inputs["Wgate"], dtype=np.float32)
    wu = np.asarray(inputs["Wup"], dtype=np.float32)
    wd = np.asarray(inputs["Wdown"], dtype=np.float32)
    wout = np.asarray(inputs["W_out"], dtype=np.float32)
    b_out = np.asarray(inputs["b_out"], dtype=np.float32)

    W = min(512, c.FF)
    wqT = tile_lhsT(_bf((wq * ln1[None, :] / math.sqrt(c.HD)).T))
    wkT = tile_lhsT(_bf((wk * ln1[None, :]).T))
    woT = tile_lhsT(_bf(wo.T))
    wvT = _bf((wv * ln1[None, :]).T)
    if c.mlp_fp8 >= 2:
        wgT = tile_fog_dr8((wg * ln2[None, :]).T, W)
        wuT = tile_fog_dr8((wu * ln2[None, :]).T, W)
    else:
        wgT = tile_fog(_bf((wg * ln2[None, :]).T), W)
        wuT = tile_fog(_bf((wu * ln2[None, :]).T), W)
    if c.mlp_fp8 >= 1:
        wdT = tile_lhsT_dr8(wd.T)
    else:
        wdT = tile_lhsT(_bf(wd.T))
    woutT = _bf(wout.T)
    bias_row = _bf(b_out[None, :])
    cos_t, sin_t, shiftT, sel = make_tables(c)
    onesb = np.ones((128, 128), dtype=np.float32).astype(NPBF)

    shared = dict(wqT=wqT, wkT=wkT, woT=woT, wvT=wvT, wgT=wgT, wuT=wuT,
                  wdT=wdT, woutT=woutT, bias_row=bias_row, cos_s=cos_t,
                  sin_s=sin_t, shiftT=shiftT, sel=sel, onesb=onesb)

    in_maps = []
    halves = c.S // c.TQ
    for core in range(n_cores):
        b, hf = core // halves, core % halves
        x_fm_f32 = np.ascontiguousarray(x[b].T)                  # [D, S]
        x_fm = x_fm_f32.astype(NPBF)
        x_own = np.ascontiguousarray(x_fm_f32[:, hf * c.TQ:(hf + 1) * c.TQ])
        m = dict(shared)
        m["x_fm"] = x_fm
        m["x_own"] = x_own
        in_maps.append(m)
    return in_maps


_NC_CACHE = {}


def _get_nc(cfg: Cfg):
    if cfg not in _NC_CACHE:
        _NC_CACHE[cfg] = build_bass(cfg)
    return _NC_CACHE[cfg]


def kernel(**inputs) -> np.ndarray:
    c = FULL
    nc = _get_nc(c)
    in_maps = prep_in_maps(c, inputs)
    res = bass_utils.run_bass_kernel_spmd(nc, in_maps, core_ids=list(range(N_CORES)))
    out = np.concatenate([res.results[i]["logits"] for i in range(N_CORES)], axis=0)
    return out.reshape(-1, 8, c.V)
